# revision 40
# baseline (speedup 1.0000x reference)
"""Bahdanau attention kernel for Trainium2 (8 NeuronCores, data-parallel over batch).

Reference computation (B=32, T=4096, D=U=512):
    q_proj = query @ W1 + b1                      [B, 1, U]
    v_proj = values @ W2 + b2                     [B, T, U]
    scores = tanh(q_proj + v_proj) @ V + bv       [B, T, 1]
    attn   = softmax(scores, axis=1)
    out    = sum(attn * values, axis=1)           [B, D]

Device strategy (per core, 4 batches), using only PE + ACT + DMA (the DVE and
the accum-out paths are unusable on this runtime):
  - Host folds b1/b2 into q_eff = query@W1 + b1 + b2, drops bv (softmax shift
    invariant), ships values twice in partition-pre-shuffled layouts so every
    device DMA is a contiguous-per-partition slab: valuesT [b,128,4,T] fp8
    (projection matmul, DoubleRow) and valuesN [b,128,32,D] bf16 (context
    matmul).
  - DMA queues are split across engines (vT/ctx-out on Sync, vN/colsums on
    GpSimd) and batch b+1's loads are emitted at the top of stage b, so
    descriptor generation and transfers run a full batch ahead of compute.
  - A memset-fed stream of dummy matmuls at t~0 warms the PE HAM clock gate
    (cold K=4/8 -> warm 8/8) while the first real DMAs are in flight; a dummy
    tanh pulls the ACT table load forward.
  - v_proj computed transposed [U, t] with W2 stationary; fp8 DoubleRow
    (2 matmuls of K=256) with W2 pre-scaled by F8_SCALE on host, un-scaled
    inside the ACT tanh (scale=1/F8_SCALE); q_eff rides the tanh bias.
  - scores: tanh tiles become the stationary operand against V [128,1], so
    scores land directly in [128, T/128] partition-major PSUM layout.
  - softmax without division or max-subtraction (|scores| <= ||V||_1 ~ 8, safe
    in fp32): unnormalized exp on device, division by the exp-sum on host via
    the colsums output. Sums via ones-matmul on PE.
  - Context: T/128 accumulating [128,1]x[128,512] matmuls with attn stationary.
"""

import os
import sys

import numpy as np

try:
    import ml_dtypes  # noqa: F401
except ImportError:  # pragma: no cover
    sys.path.insert(0, "/opt/trn_rl_repo")
    import ml_dtypes  # noqa: F401

try:
    import concourse  # noqa: F401
except ImportError:  # pragma: no cover
    sys.path.insert(0, "/opt/trn_rl_repo")

BF16 = np.dtype(ml_dtypes.bfloat16)
FP8 = np.dtype(ml_dtypes.float8_e4m3)

B, T, D, U = 32, 4096, 512, 512
N_CORES = 8
BPC = B // N_CORES  # batches per core

F8_SCALE = 64.0  # host scales W2 by this; ACT tanh un-scales via scale=1/F8_SCALE

MODE = os.environ.get("BAHDANAU_MODE", "fp8")  # "fp8" | "bf16"

_MODULES: dict = {}


def _build(bpc: int = BPC, t: int = T, mode: str = "fp8"):
    """Build + compile the per-core Bass module. Shapes are per-core shards."""
    from contextlib import ExitStack

    import concourse.bass as bass
    import concourse.tile as tile
    from concourse import bacc, mybir

    f32 = mybir.dt.float32
    bf16 = mybir.dt.bfloat16
    fp8 = mybir.dt.float8e4
    FT = mybir.ActivationFunctionType
    PSUM = bass.MemorySpace.PSUM
    DR = mybir.MatmulPerfMode.DoubleRow

    use_fp8 = mode == "fp8"
    vt_dt = fp8 if use_fp8 else bf16
    tb_n = t // 128  # 128-row t-blocks per batch
    tc_n = t // 512  # 512-col t-chunks per batch
    tanh_scale = (1.0 / F8_SCALE) if use_fp8 else 1.0

    nc = bacc.Bacc(
        "TRN2", target_bir_lowering=False, debug=False, enable_asserts=False
    )

    # Pre-shuffled layouts: leading 128 dim is the SBUF partition, all device
    # DMAs are contiguous-per-partition slabs.
    vT_d = nc.dram_tensor("valuesT", [bpc, 128, 4, t], vt_dt, kind="ExternalInput")
    vN_d = nc.dram_tensor("valuesN", [bpc, 128, tb_n, D], bf16, kind="ExternalInput")
    w2_d = nc.dram_tensor("w2t", [128, 4, U], vt_dt, kind="ExternalInput")
    vcc_d = nc.dram_tensor("vcc", [128, 5], bf16, kind="ExternalInput")
    qe_d = nc.dram_tensor("q_eff", [128, bpc, 4], f32, kind="ExternalInput")
    out_d = nc.dram_tensor("ctx_out", [bpc, 4, D], f32, kind="ExternalOutput")
    cols_d = nc.dram_tensor("colsums", [bpc, 128, tb_n], bf16, kind="ExternalOutput")

    with tile.TileContext(nc) as tc, ExitStack() as ctx:
        const = ctx.enter_context(tc.tile_pool(name="const", bufs=1))
        vT_pool = ctx.enter_context(tc.tile_pool(name="vT", bufs=2))
        vN_pool = ctx.enter_context(tc.tile_pool(name="vN", bufs=2))
        tanh_pool = ctx.enter_context(tc.tile_pool(name="tanh", bufs=16))
        attn_pool = ctx.enter_context(tc.tile_pool(name="attn", bufs=3))
        ctxs_pool = ctx.enter_context(tc.tile_pool(name="ctxs", bufs=2))
        vp_psum = ctx.enter_context(tc.tile_pool(name="vp_ps", bufs=3, space=PSUM))
        sco_psum = ctx.enter_context(tc.tile_pool(name="sc_ps", bufs=1, space=PSUM))
        ctx_psum = ctx.enter_context(tc.tile_pool(name="ctx_ps", bufs=1, space=PSUM))

        # --- consts first (tiny, needed by the first matmul/tanh), then the
        # first batch's vT. Everything loads through the single Sync HW-DGE
        # queue in priority order: splitting loads across queues just steals
        # HBM bandwidth from the critical path (per-core ~360 GB/s ceiling).
        vT_tiles: dict = {}

        def prefetch_vT(b, chunks):
            vT_sb = vT_pool.tile([128, 4, t], vt_dt)
            vT_tiles[b] = vT_sb
            lo = 0
            for cs in chunks:
                sl = slice(lo, lo + cs)
                nc.sync.dma_start(vT_sb[:, :, sl], vT_d[b][:, :, sl])
                lo += cs

        w2_sb = const.tile([128, 4, U], vt_dt)
        nc.sync.dma_start(w2_sb[:], w2_d.ap())
        # first projection matmul needs only w2 + the first 512 t-columns:
        # get that chunk moving before the remaining consts
        prefetch_vT(0, [512])
        vcc_sb = const.tile([128, 5], bf16)
        nc.sync.dma_start(vcc_sb[:], vcc_d.ap())
        qe_sb = const.tile([128, bpc, 4], f32)
        nc.sync.dma_start(qe_sb[:], qe_d.ap())
        vT0 = vT_tiles[0]
        for sl in (slice(512, 1024), slice(1024, 2048), slice(2048, 4096)):
            nc.sync.dma_start(vT0[:, :, sl], vT_d[0][:, :, sl])

        # --- PE/ACT warm-up: dummy matmuls unthrottle the HAM clock gate and
        # a dummy tanh pulls the ACT table load forward, all while the first
        # real DMAs are still in flight ---
        warm_sb = const.tile([128, 128], bf16)
        nc.gpsimd.memset(warm_sb[:], 0)
        warm_act = const.tile([128, 1], bf16)
        nc.scalar.activation(warm_act[:], warm_sb[:, 0:1], FT.Tanh)
        warm_ps = sco_psum.tile([128, tb_n], f32, tag="sco")
        for _ in range(150):
            nc.tensor.matmul(
                warm_ps[:], warm_sb[:], warm_sb[:, :tb_n], start=True, stop=True
            )

        def stage(b, prev_tail):
            """Full per-batch pipeline: project, scores, exp, context.

            The previous batch's final-pair context/sum work (`prev_tail`) is
            emitted after this batch's first pair so the PE never stalls the
            ACT pipeline at batch boundaries. Returns this batch's tail."""
            n_pairs = tc_n // 2
            vT_sb = vT_tiles[b]
            vN_sb = vN_pool.tile([128, tb_n, D], bf16)
            scoresP = sco_psum.tile([128, tb_n], f32, tag="sco")
            expP = attn_pool.tile([128, tb_n], bf16)
            cps = ctx_psum.tile([128, D], f32)
            # this batch's vN first (needed within ~1 pair), then next
            # batch's vT — the Sync queue drains strictly in order
            for pq in range(4):
                sl = slice(pq * 8, (pq + 1) * 8)
                nc.sync.dma_start(vN_sb[:, sl, :], vN_d[b][:, sl, :])
            if b + 1 < bpc:
                prefetch_vT(b + 1, [2048, 2048])

            def ctx_mms(pair):
                # 4x column-tiled: strips at array columns 0/32/64/96 run
                # concurrently, accumulating into psum partitions 0/32/64/96
                for k in range(8):
                    n = pair * 8 + k
                    g = n % 4
                    nc.tensor.matmul(
                        cps[32 * g : 32 * g + 1, :],
                        expP[:, n : n + 1],
                        vN_sb[:, n, :],
                        start=(n < 4),
                        stop=(n >= tb_n - 4),
                        tile_position=(0, 32 * g),
                    )

            tanh_by_pair: dict = {}

            def scores_mms(p):
                for tl8 in range(8):
                    blk = p * 8 + tl8
                    for ub in range(4):
                        nc.tensor.matmul(
                            scoresP[:, blk : blk + 1],
                            tanh_by_pair[p][ub][:, tl8 // 4, bass.ts(tl8 % 4, 128)],
                            vcc_sb[:, ub : ub + 1],
                            start=(ub == 0),
                            stop=(ub == 3),
                        )

            def exp_act(p):
                nc.scalar.activation(
                    expP[:, p * 8 : (p + 1) * 8],
                    scoresP[:, p * 8 : (p + 1) * 8],
                    FT.Exp,
                )

            for pair in range(n_pairs):
                tanh_tiles = []
                for ub in range(4):
                    vp = vp_psum.tile([128, 2, 512], f32)
                    # j outer / half inner so consecutive matmuls share the
                    # same stationary W2 block (LDWEIGHTS amortization)
                    if use_fp8:
                        for j in range(2):
                            for half in range(2):
                                tc8 = pair * 2 + half
                                nc.tensor.matmul(
                                    vp[:, half, :],
                                    w2_sb[:, 2 * j : 2 * j + 2, bass.ts(ub, 128)],
                                    vT_sb[:, 2 * j : 2 * j + 2, bass.ts(tc8, 512)],
                                    start=(j == 0),
                                    stop=(j == 1),
                                    perf_mode=DR,
                                )
                    else:
                        for j in range(4):
                            for half in range(2):
                                tc8 = pair * 2 + half
                                nc.tensor.matmul(
                                    vp[:, half, :],
                                    w2_sb[:, j, bass.ts(ub, 128)],
                                    vT_sb[:, j, bass.ts(tc8, 512)],
                                    start=(j == 0),
                                    stop=(j == 3),
                                )
                    th = tanh_pool.tile([128, 2, 512], bf16)
                    nc.scalar.activation(
                        th[:],
                        vp[:],
                        FT.Tanh,
                        bias=qe_sb[:, b, ub : ub + 1],
                        scale=tanh_scale,
                    )
                    tanh_tiles.append(th)
                tanh_by_pair[pair] = tanh_tiles
                # Software pipeline: this pair's projection is emitted first,
                # then work whose dependencies resolved a pair (or more) ago —
                # scores/exp of pair-1, ctx of pair-2 — so the PE queue never
                # head-of-line-blocks on a fresh tanh and the ACT engine
                # (the steady-state bottleneck) stays saturated. The previous
                # batch's deferred tail is split: its scores/exp at pair 0
                # (so its exp sits only one tanh-quad deep in the ACT FIFO),
                # its ctx/copy at pair 1 (by then that exp has drained).
                if pair == 0 and prev_tail is not None:
                    prev_tail[0]()
                if pair == 1 and prev_tail is not None:
                    prev_tail[1]()
                if pair >= 1:
                    scores_mms(pair - 1)
                    exp_act(pair - 1)
                if pair >= 2:
                    ctx_mms(pair - 2)

            def tail_a():
                scores_mms(n_pairs - 1)
                exp_act(n_pairs - 1)

            def tail_b():
                ctx_mms(n_pairs - 2)
                ctx_mms(n_pairs - 1)
                # softmax denominator: ship the bf16 exp weights and sum on
                # the host — numerically identical to a device-side ones-
                # matmul over the same tile, and off the critical path
                nc.gpsimd.dma_start(cols_d[b], expP[:])
                cs_raw = ctxs_pool.tile([128, D], f32)
                nc.scalar.copy(cs_raw[:], cps[:])
                nc.sync.dma_start(out_d[b], cs_raw[0:97:32, :])

            return (tail_a, tail_b)

        pend = None
        for b in range(bpc):
            pend = stage(b, pend)
        pend[0]()
        pend[1]()

    nc.compile()
    return nc


def _get_module(bpc: int = BPC, t: int = T, mode: str | None = None):
    mode = MODE if mode is None else mode
    key = (mode, bpc, t)
    if key not in _MODULES:
        _MODULES[key] = _build(bpc, t, mode)
    return _MODULES[key]


def _prep_inputs(query, values, W1, b1, W2, b2, V, bv, mode: str | None = None):
    """Host-side preprocessing: fold biases, cast, pre-shuffle, shard."""
    mode = MODE if mode is None else mode
    query = np.asarray(query, np.float32)
    values = np.asarray(values, np.float32)
    W1 = np.asarray(W1, np.float32)
    b1 = np.asarray(b1, np.float32)
    W2 = np.asarray(W2, np.float32)
    b2 = np.asarray(b2, np.float32)
    V = np.asarray(V, np.float32)

    q_eff = (
        query.astype(np.float64) @ W1.astype(np.float64)
        + b1.astype(np.float64)
        + b2.astype(np.float64)
    ).astype(np.float32)  # [B, U]; bv dropped (softmax shift invariance)

    # valuesN [B, 128, T/128, D]: vN[b, p, n, d] = values[b, n*128+p, d]
    vN = np.ascontiguousarray(
        values.reshape(B, T // 128, 128, D).transpose(0, 2, 1, 3)
    ).astype(BF16)
    # valuesT [B, 128, 4, T]: vT[b, p, db, t] = values[b, t, db*128+p]
    vTf = values.transpose(0, 2, 1)  # [B, D, T]
    vTp = np.ascontiguousarray(vTf.reshape(B, 4, 128, T).transpose(0, 2, 1, 3))
    if mode == "fp8":
        vT = vTp.astype(FP8)
        w2s = (W2 * F8_SCALE).astype(FP8)
    else:
        vT = vTp.astype(BF16)
        w2s = W2.astype(BF16)
    # w2 [128, 4, U]: w2[p, db, u] = W2[db*128+p, u]
    w2 = np.ascontiguousarray(w2s.reshape(4, 128, U).transpose(1, 0, 2))
    # vcc [128, 5]: cols 0-3 = V[ub*128+p], col 4 = ones
    vcc = np.ones((128, 5), np.float32)
    vcc[:, :4] = V.reshape(4, 128).T
    vcc = vcc.astype(BF16)
    # q_eff [128, B, 4]: qe[p, b, ub] = q_eff[b, ub*128+p]
    qe = np.ascontiguousarray(q_eff.reshape(B, 4, 128).transpose(2, 0, 1))

    in_maps = []
    for c in range(N_CORES):
        s = slice(c * BPC, (c + 1) * BPC)
        in_maps.append(
            {
                "valuesT": vT[s],
                "valuesN": vN[s],
                "w2t": w2,
                "vcc": vcc,
                "q_eff": qe[:, s, :],
            }
        )
    return in_maps


def _run(in_maps, trace=False, mode: str | None = None, **kw):
    from concourse.bass_utils import run_bass_kernel_spmd

    nc = _get_module(mode=mode)
    res = run_bass_kernel_spmd(
        nc, in_maps, core_ids=list(range(N_CORES)), trace=trace, **kw
    )
    raw = np.concatenate(
        [np.asarray(res.results[c]["ctx_out"]) for c in range(N_CORES)], axis=0
    ).astype(np.float32).sum(axis=1)  # collapse the 4 col-tile strips
    sums = np.concatenate(
        [np.asarray(res.results[c]["colsums"]) for c in range(N_CORES)], axis=0
    ).astype(np.float32)
    out = raw / sums.sum(axis=(1, 2), keepdims=False).reshape(-1, 1)
    return out, res


def kernel(query, values, W1, b1, W2, b2, V, bv):
    in_maps = _prep_inputs(query, values, W1, b1, W2, b2, V, bv)
    out, _ = _run(in_maps, trace=False)
    return out
